# revision 61
# baseline (speedup 1.0000x reference)
"""EnhancedGradientConsistencyLoss on 8 TRN2 NeuronCores.

Strategy: pure data parallel over batch B=8 (1 image per core).
Per core (inputs [3,512,512]):
  - vertical 3-tap sobel + 9-tap gaussian as banded block matmuls on PE,
    fed f32r (fp32 at bf16 rate, out free >= 256); band consts dedup'd to
    11 distinct 128x128 blocks; psum in 4x 2-block tiles for turnaround;
    PE pstate pre-warmed with dummy matmuls on a memset scratch tile
  - mask loaded via gpsimd casting DMA (int32 dram -> bf16 sbuf), issued
    late so it cannot delay the x/t loads on the shared DMA engines;
    x/t loaded as 2-block half tiles so the first conv matmuls start
    after half a DMA instead of a full one
  - elementwise math on DVE as bf16 TensorTensor (2x mode) /
    tensor_scalar (4x mode); scalar_tensor_tensor only where its fused
    accumulator is needed (no fast mode there)
  - four custom DVE ops registered into concourse.dve_ops at import:
      SUMSQ_ANT:      gx^2 + gy^2 in one pass
      RATIO_ANT:      (h-d)*recip_1NR(h+d), bitwise-NOT seeded reciprocal
      ABSMUL_RED_ANT: |mago-magt|*w with fused sum accumulation
      AFFINE2_ANT:    p1*R1 + p2*R2 (gauss tap combine)
    (theta = 2*atan(sqrt((h-d)/(h+d))), h = mago*magt, d = dot(go, gt))
  - boundary weight w = 1 - |2*g - 1| (clip-free: g in [0,1]); horizontal
    gauss truncated to +-2 taps, renormalized (<0.9% kernel mass)
  - gauss pair-adds on Pool; psum evacuations on ACT (gpsimd cannot
    touch PSUM), triple-buffered evac tiles; software pipeline runs
    load/conv one channel ahead of compute
  - reductions fused via accum_out -> [128,16] partials; host combines
ACT tables: one sqrt-set phase (Copy/Sqrt/Abs) + one trig phase (Arctan)
behind a scheduler-only fence -> 3 table loads total; the trig tail is
half-split (atan + S2 per 2-block half) for finer ACT/DVE overlap.
TimelineSim: 98.5us (baseline 272.5us, 2.77x); rel err 3.3e-3 (tol 2e-2).
"""

import math
import os
import sys

import numpy as np

sys.path.insert(0, "/opt/trn_rl_repo")

import concourse.bass as bass  # noqa: E402
import concourse.bacc as bacc  # noqa: E402
import concourse.tile as tile  # noqa: E402
from concourse import mybir  # noqa: E402
from concourse.bass_utils import run_bass_kernel_spmd  # noqa: E402

F32 = mybir.dt.float32
F32R = mybir.dt.float32r
BF16 = mybir.dt.bfloat16
I32 = mybir.dt.int32
AF = mybir.ActivationFunctionType
OP = mybir.AluOpType

C, H, W = 3, 512, 512
NB = 4          # H blocks of 128
P = 128
N_CORES = 8
HALO = 2        # mv halo cols each side (taps +-2)
WTM = W + 2 * HALO
WTS = W + 2    # sobel halo 1

EPS_MAG = 1e-8


def _gauss_kernel_np():
    r = 4
    x = np.arange(-r, r + 1, dtype=np.float64)
    k = np.exp(-0.5 * x * x)
    return k / k.sum()


K9 = _gauss_kernel_np()
KEPT_MASS = 1.0 - 2.0 * (K9[0] + K9[1])
R1 = float(K9[5] / K9[4])
R2 = float(K9[6] / K9[4])
R3 = float(K9[7] / K9[4])
S_YW = float(-2.0 * K9[4] / KEPT_MASS)   # yw = |S_YW * a3 + 1|


def _full_band_matrices():
    """A_smooth/A_diff (zero pad), A_gauss (symmetric pad), each [H, H] with
    out = A @ x along the H axis."""
    As = np.zeros((H, H), np.float64)
    Ad = np.zeros((H, H), np.float64)
    for h in range(H):
        for d, kv in ((-1, 1.0), (0, 2.0), (1, 1.0)):
            s = h + d
            if 0 <= s < H:
                As[h, s] += kv
        for d, kv in ((-1, -1.0), (1, 1.0)):
            s = h + d
            if 0 <= s < H:
                Ad[h, s] += kv
    Ag = np.zeros((H, H), np.float64)
    for h in range(H):
        for d in range(-4, 5):
            s = h + d
            if s < 0:
                s = -s - 1
            elif s > H - 1:
                s = 2 * H - 1 - s
            Ag[h, s] += K9[d + 4]
    return As, Ad, Ag


# per conv: list of (dst_block i, src_block j); diag first per bank so the
# first matmul into each psum bank carries start=True.
_BLOCKS = []
for i in range(NB):
    _BLOCKS.append((i, i))
    if i > 0:
        _BLOCKS.append((i, i - 1))
    if i < NB - 1:
        _BLOCKS.append((i, i + 1))
N_BLK = len(_BLOCKS)  # 10


def _consts_arrays():
    """Dedup: As/Ad have 3 distinct blocks each (diag/up/down); Ag has 5
    (diag top/mid/bot + up/down). Layout: cf = [As d,u,dn | Ad d,u,dn],
    cg = [G top, mid, bot, up, down]."""
    As, Ad, Ag = _full_band_matrices()

    def b(A, i, j):
        return A[i * P:(i + 1) * P, j * P:(j + 1) * P].T.astype(np.float32)

    f_blocks = []
    for A in (As, Ad):
        f_blocks += [b(A, 0, 0), b(A, 1, 0), b(A, 0, 1)]
    g_blocks = [b(Ag, 0, 0), b(Ag, 1, 1), b(Ag, 3, 3), b(Ag, 1, 0), b(Ag, 1, 2)]
    return np.concatenate(f_blocks, axis=1), np.concatenate(g_blocks, axis=1)


# --- custom DVE ops (registered into concourse.dve_ops at import) ---------
from concourse import dve_ops as _dvo  # noqa: E402
from concourse.dve_spec import (  # noqa: E402
    Spec as _Spec, Src0 as _S0, Src1 as _S1, C0 as _C0, C1 as _C1,
    Bin as _Bin, AluOp as _AluOp, sq as _sq, lower as _lower,
    _has_src1 as _has_src1,
)
from concourse.dve_uop import DveOpSpec as _DveOpSpec  # noqa: E402
from concourse.dve_table_gen import dve_ver_for as _dve_ver_for  # noqa: E402

# matches RECIP_APPROX_FAST_CONSTS seed/NR constants (1 NR pass, ~0.4% rel)
_RC0 = -0.23549792
_RC1 = 2.0017324


def _register_dve_op(name, body, reference, accum=None, accum_init=None):
    if name in _dvo._SUB_OPCODE_FOR_NAME:
        for op in _dvo.OPS:
            if op.name == name:
                return op
    row = max(_dvo._SUB_OPCODE_FOR_NAME.values()) + 1
    assert row < 0x20
    _dvo._SUB_OPCODE_FOR_NAME[name] = row
    if accum is not None:
        spec = _Spec(body=body, reference=reference, accum=accum,
                     accum_init=accum_init)
    else:
        spec = _Spec(body=body, reference=reference)
    shas = {}
    for ver in ("v3", "v4"):
        try:
            uops = _lower(spec, ver=ver)
            shas[ver] = _DveOpSpec(
                name=name, opcode=row, uops=uops, rd1_en=_has_src1(spec)
            ).sha(ver)
        except Exception:
            pass
    op = _dvo.DveOp(name=name, spec=spec, subdim=False, uops_sha=shas)
    _dvo.OPS.append(op)
    _dvo.CUSTOM_DVE_SPECS[name] = spec
    return op


def _ratio_ref(in0, in1, s0, s1, imm2):
    s = (in0 + in1).astype(np.float32)
    not_s = (~s.view(np.int32)).view(np.float32)
    y0 = not_s * np.float32(s0)
    y1 = y0 * (np.float32(s1) - s * y0)
    return ((in0 - in1) * y1).astype(np.float32)


_rs = _S0 + _S1
_rn = _Bin(_AluOp.BITWISE_NOT, _rs, _rs)
_ry0 = _rn * _C0
_ry1 = _ry0 * (_C1 - _rs * _ry0)
RATIO_ANT = _register_dve_op(
    "RATIO_ANT", (_S0 - _S1) * _ry1, _ratio_ref)

from operator import add as _opadd  # noqa: E402
from concourse.dve_spec import Zero as _Zero  # noqa: E402


def _absmul_ref(in0, in1, s0, s1, imm2):
    b = (np.abs(in0.astype(np.float32)) * in1).astype(np.float32)
    return b, b.reshape(b.shape[0], -1).sum(axis=-1, keepdims=True)


ABSMUL_RED_ANT = _register_dve_op(
    "ABSMUL_RED_ANT", _Bin(_AluOp.ABSOLUTE_VALUE, _S0, _S0) * _S1,
    _absmul_ref, accum=_opadd, accum_init=_Zero)

AFFINE2_ANT = _register_dve_op(
    "AFFINE2_ANT", _S0 * _C0 + _S1 * _C1,
    lambda in0, in1, s0, s1, imm2: (
        in0.astype(np.float32) * np.float32(s0)
        + in1.astype(np.float32) * np.float32(s1)))

SUMSQ_ANT = _register_dve_op(
    "SUMSQ_ANT", _sq(_S0) + _sq(_S1),
    lambda in0, in1, s0, s1, imm2: (
        in0.astype(np.float32) ** 2 + in1.astype(np.float32) ** 2))


CONSTS_F, CONSTS_G32 = _consts_arrays()
import ml_dtypes  # noqa: E402
CONSTS_G = CONSTS_G32.astype(ml_dtypes.bfloat16)
CF_W = CONSTS_F.shape[1]   # 20*128
CG_W = CONSTS_G.shape[1]   # 10*128


def _emit(tc, partials, o_dram, t_dram, m_dram, cf_dram, cg_dram):
    nc = tc.nc
    from contextlib import ExitStack
    stack = ExitStack()

    consts_pool = stack.enter_context(tc.tile_pool(name="consts", bufs=1))
    in_pool = stack.enter_context(tc.tile_pool(name="inp", bufs=1))
    evac = stack.enter_context(tc.tile_pool(name="evac", bufs=1))
    work = stack.enter_context(tc.tile_pool(name="work", bufs=1))
    ret = stack.enter_context(tc.tile_pool(name="ret", bufs=1))
    psum = stack.enter_context(tc.tile_pool(name="psum", bufs=2, space="PSUM"))
    outp = stack.enter_context(tc.tile_pool(name="outp", bufs=1))

    cf = consts_pool.tile([P, CF_W], F32R)
    nc.gpsimd.dma_start(out=cf[:], in_=cf_dram)
    cg = consts_pool.tile([P, CG_W], BF16)
    nc.gpsimd.dma_start(out=cg[:], in_=cg_dram)

    ptile = outp.tile([P, 16], F32)
    nc.vector.memset(ptile[:], 0.0)

    biases = outp.tile([P, 3], F32)
    nc.vector.memset(biases[:, 0:1], EPS_MAG)
    nc.vector.memset(biases[:, 1:2], 1.0)
    nc.vector.memset(biases[:, 2:3], 0.0)
    b_eps = biases[:, 0:1]
    b_one = biases[:, 1:2]
    b_zero = biases[:, 2:3]

    def band_f(conv_idx, ij):
        i, j = ij
        kind = 0 if j == i else (1 if j == i - 1 else 2)
        base = (conv_idx * 3 + kind) * P
        return cf[:, base:base + P]

    def band_g(ij):
        i, j = ij
        if j == i:
            kind = 0 if i == 0 else (2 if i == NB - 1 else 1)
        else:
            kind = 3 if j == i - 1 else 4
        return cg[:, kind * P:kind * P + P]

    def vconv(band, src, evac_fn):
        """10 block matmuls band x src, in 2-block halves -> finer psum
        turnaround (4 x 4KB psum tiles in flight)."""
        for h in (0, 1):
            ps = psum.tile([P, 2, W], F32, tag="ps", name="pst", bufs=4)
            for k, i in enumerate((2 * h, 2 * h + 1)):
                touched = [ij for ij in _BLOCKS if ij[0] == i]
                for n, (ii, jj) in enumerate(touched):
                    nc.tensor.matmul(
                        ps[:, k, :], band((ii, jj)), src(jj),
                        start=(n == 0), stop=(n == len(touched) - 1),
                    )
            evac_fn(ps, h)

    # retained across phases, per channel
    qR = [ret.tile([P, NB, W], BF16, tag=f"q{c}", name=f"qr{c}") for c in range(C)]
    wR = [ret.tile([P, NB, W], BF16, tag=f"w{c}", name=f"wr{c}") for c in range(C)]

    # per-channel state handed from the load/conv stage to the compute stage
    st = [None] * C

    def stage_load_conv(c):
        xa = in_pool.tile([P, 2, W], F32R, tag="xa", bufs=2)
        xb = in_pool.tile([P, 2, W], F32R, tag="xb", bufs=2)
        ta_ = in_pool.tile([P, 2, W], F32R, tag="ta", bufs=2)
        tb = in_pool.tile([P, 2, W], F32R, tag="tb", bufs=2)
        xr = o_dram[c].rearrange("(b p) w -> p b w", p=P)
        tr = t_dram[c].rearrange("(b p) w -> p b w", p=P)
        nc.sync.dma_start(out=xa[:], in_=xr[:, 0:2, :])
        nc.sync.dma_start(out=xb[:], in_=xr[:, 2:4, :])
        nc.sync.dma_start(out=ta_[:], in_=tr[:, 0:2, :])
        nc.sync.dma_start(out=tb[:], in_=tr[:, 2:4, :])

        def xsrc(j):
            return xa[:, j, :] if j < 2 else xb[:, j - 2, :]

        def tsrc(j):
            return ta_[:, j, :] if j < 2 else tb[:, j - 2, :]
        mf = in_pool.tile([P, NB, W], BF16, tag="mf", bufs=2)

        sv = evac.tile([P, NB, WTS], BF16, tag="sv", bufs=3)
        sd = evac.tile([P, NB, WTS], BF16, tag="sd", bufs=3)
        tv = evac.tile([P, NB, WTS], BF16, tag="tv", bufs=3)
        td = evac.tile([P, NB, WTS], BF16, tag="td", bufs=3)
        mv = evac.tile([P, NB, WTM], BF16, tag="mv", bufs=3)

        # zero sobel halos (cheap; keeps zero-pad conv semantics)
        for t in (sv, sd, tv, td):
            nc.gpsimd.memset(t[:, :, 0:1], 0.0)
            nc.gpsimd.memset(t[:, :, W + 1:W + 2], 0.0)

        def ev(dst, off):
            return lambda ps, h: nc.scalar.copy(
                out=dst[:, 2 * h:2 * h + 2, off:off + W], in_=ps[:])

        vconv(lambda b: band_f(0, b), xsrc, ev(sv, 1))
        vconv(lambda b: band_f(1, b), xsrc, ev(sd, 1))
        vconv(lambda b: band_f(0, b), tsrc, ev(tv, 1))
        vconv(lambda b: band_f(1, b), tsrc, ev(td, 1))
        nc.gpsimd.dma_start(out=mf[:], in_=m_dram[c].rearrange("(b p) w -> p b w", p=P))
        vconv(band_g, lambda j: mf[:, j, :], ev(mv, HALO))

        # reflect halo for mv: m[-1-k] = m[k]
        for k in range(HALO):
            nc.gpsimd.tensor_copy(
                out=mv[:, :, HALO - 1 - k:HALO - k],
                in_=mv[:, :, HALO + k:HALO + k + 1],
            )
            nc.gpsimd.tensor_copy(
                out=mv[:, :, HALO + W + k:HALO + W + k + 1],
                in_=mv[:, :, HALO + W - 1 - k:HALO + W - k],
            )
        st[c] = (sv, sd, tv, td, mv)

    def wt(tag):
        return work.tile([P, NB, W], BF16, tag=tag, bufs=2, name=f"wk_{tag}")

    def stage_compute(c):
        sv, sd, tv, td, mv = st[c]
        stt = nc.vector.scalar_tensor_tensor
        ts = nc.vector.tensor_scalar
        tadd = nc.vector.tensor_add
        tsub = nc.vector.tensor_sub
        tmul = nc.vector.tensor_mul
        tdiv = lambda out, in0, in1: nc.vector.tensor_tensor(
            out=out, in0=in0, in1=in1, op=OP.divide)

        def s0(t):  # sobel tile shifted -1 / 0 / +1
            return t[:, :, 0:W]

        def s1(t):
            return t[:, :, 1:W + 1]

        def s2(t):
            return t[:, :, 2:W + 2]

        def mvs(d):  # mv shifted by d
            return mv[:, :, HALO + d:HALO + W + d]

        gx = wt("gx")
        tsub(out=gx[:], in0=s2(sv), in1=s0(sv))
        gy = wt("gy")
        pair_add = tadd if c == 0 else nc.gpsimd.tensor_add
        pair_add(out=gy[:], in0=s0(sd), in1=s2(sd))
        gyc = wt("sc")
        ts(out=gyc[:], in0=s1(sd), scalar1=2.0, scalar2=None, op0=OP.mult)
        tadd(out=gy[:], in0=gy[:], in1=gyc[:])
        hx = wt("hx")
        tsub(out=hx[:], in0=s2(tv), in1=s0(tv))
        hy = wt("hy")
        pair_add(out=hy[:], in0=s0(td), in1=s2(td))
        hyc = wt("sc")
        ts(out=hyc[:], in0=s1(td), scalar1=2.0, scalar2=None, op0=OP.mult)
        tadd(out=hy[:], in0=hy[:], in1=hyc[:])

        # dot products
        d1 = wt("d1")
        tmul(out=d1[:], in0=gx[:], in1=hx[:])
        d2 = wt("d2")
        tmul(out=d2[:], in0=gy[:], in1=hy[:])
        tadd(out=d1[:], in0=d1[:], in1=d2[:])
        dd = d1

        # squared magnitudes via fused custom op: a2 = gx^2 + gy^2
        a2m = wt("sy")
        nc.vector._custom_dve(SUMSQ_ANT, out=a2m[:], in0=gx[:], in1=gy[:])
        mago = wt("mago")
        nc.scalar.activation(mago[:], a2m[:], AF.Sqrt, bias=b_eps)
        b2m = wt("sy")
        nc.vector._custom_dve(SUMSQ_ANT, out=b2m[:], in0=hx[:], in1=hy[:])
        magt = wt("mago")
        nc.scalar.activation(magt[:], b2m[:], AF.Sqrt, bias=b_eps)

        # h = mago*magt; r = (h-d)/(h+d) fused (1-NR recip); q = sqrt(clamp(r))
        hh = wt("gy")
        tmul(out=hh[:], in0=mago[:], in1=magt[:])
        u = wt("hy")
        nc.vector._custom_dve(RATIO_ANT, out=u[:], in0=hh[:], in1=dd[:],
                              s0=_RC0, s1=_RC1)
        ts(out=u[:], in0=u[:], scalar1=1e30, scalar2=0.0, op0=OP.min, op1=OP.max)
        nc.scalar.activation(qR[c][:], u[:], AF.Sqrt, bias=b_zero)

        # |mago-magt| stored into magt (scratch)
        tsub(out=magt[:], in0=mago[:], in1=magt[:])
        dmg = magt

        # horizontal gauss on mv (taps +-3, renormalized); pairs on Pool
        p1 = wt("d1")
        nc.gpsimd.tensor_add(out=p1[:], in0=mvs(-1), in1=mvs(1))
        p2 = wt("d2")
        nc.gpsimd.tensor_add(out=p2[:], in0=mvs(-2), in1=mvs(2))
        nc.vector._custom_dve(AFFINE2_ANT, out=p2[:], in0=p1[:], in1=p2[:],
                              s0=R1, s1=R2)
        tadd(out=p2[:], in0=p2[:], in1=mvs(0))
        a3 = p2

        # yw = |S_YW*a3 + 1|, accumulate sum(yw); w = 1 - yw
        yw = wt("hx")
        nc.scalar.activation(yw[:], a3[:], AF.Abs, bias=b_one, scale=S_YW,
                             accum_out=ptile[:, 6 + c:7 + c])
        nc.scalar.activation(wR[c][:], yw[:], AF.Copy, bias=1.0, scale=-1.0)

        # S1 += |dmg| * w  (fused custom: |Src0|*Src1 with accumulate)
        nc.vector._custom_dve(ABSMUL_RED_ANT, out=dmg[:], in0=dmg[:],
                              in1=wR[c][:], accum_out=ptile[:, 0 + c:1 + c])

    # PE pstate warmup: fat dummy matmuls on a memset scratch tile (no
    # dependency on any DMA) so PE is at full clock when real convs start
    wsrc = work.tile([P, NB, W], BF16, tag="warm", bufs=1, name="wk_warm")
    nc.vector.memset(wsrc[:], 0.0)
    warm = psum.tile([P, 2, W], F32, tag="ps", name="warm", bufs=4)
    for r in range(8):
        nc.tensor.matmul(warm[:, r % 2, :], wsrc[:, 0, 0:P], wsrc[:, r % NB, :],
                         start=True, stop=True)

    # software pipeline: load/conv runs one channel ahead of compute
    stage_load_conv(0)
    for c in range(C):
        if c + 1 < C:
            stage_load_conv(c + 1)
        stage_compute(c)

    # trig phase: one table switch, then atan + S2 accumulation
    tc.no_sync_barrier()
    tas = []
    for c in range(C):
        ta = wt(["sy", "mago", "gy"][c])
        for h in (0, 1):
            nc.scalar.activation(ta[:, 2 * h:2 * h + 2, :],
                                 qR[c][:, 2 * h:2 * h + 2, :],
                                 AF.Arctan, bias=b_zero)
        tas.append(ta)
    for c in range(C):
        for h in (0, 1):
            nc.vector.scalar_tensor_tensor(
                out=tas[c][:, 2 * h:2 * h + 2, :],
                in0=tas[c][:, 2 * h:2 * h + 2, :], scalar=2.0,
                in1=wR[c][:, 2 * h:2 * h + 2, :],
                op0=OP.mult, op1=OP.mult,
                accum_out=ptile[:, 9 + 2 * c + h:10 + 2 * c + h])

    nc.sync.dma_start(out=partials, in_=ptile[:])
    stack.close()


_CACHED = None


def _build():
    global _CACHED
    if _CACHED is not None:
        return _CACHED
    nc = bacc.Bacc(
        "TRN2", target_bir_lowering=False, debug=False, num_devices=1
    )
    o = nc.dram_tensor("output", [C, H, W], F32R, kind="ExternalInput").ap()
    t = nc.dram_tensor("target", [C, H, W], F32R, kind="ExternalInput").ap()
    m = nc.dram_tensor("mask", [C, H, W], I32, kind="ExternalInput").ap()
    cf = nc.dram_tensor("consts_f", [P, CF_W], F32R, kind="ExternalInput").ap()
    cg = nc.dram_tensor("consts_g", [P, CG_W], BF16, kind="ExternalInput").ap()
    pout = nc.dram_tensor("partials", [P, 16], F32, kind="ExternalOutput").ap()
    with tile.TileContext(nc) as tc:
        _emit(tc, pout, o, t, m, cf, cg)
    nc.compile()
    _CACHED = nc
    return nc


def _run(output, target, mask, trace=False):
    nc = _build()
    in_maps = []
    for k in range(N_CORES):
        in_maps.append({
            "output": np.ascontiguousarray(output[k], dtype=np.float32),
            "target": np.ascontiguousarray(target[k], dtype=np.float32),
            "mask": np.ascontiguousarray(mask[k], dtype=np.int32),
            "consts_f": CONSTS_F,
            "consts_g": CONSTS_G,
        })
    res = run_bass_kernel_spmd(nc, in_maps, core_ids=list(range(N_CORES)), trace=trace)
    return res


def _combine(res):
    parts = np.stack([np.asarray(r["partials"], dtype=np.float64)
                      for r in res.results])  # [8,128,16]
    mag_sum = parts[:, :, 0:3].sum()
    dir_sum = parts[:, :, 9:15].sum()
    n = 8.0 * C * H * W
    wsum = n - parts[:, :, 6:9].sum()
    mag_mean = mag_sum / n
    if wsum > 0:
        mag_loss = mag_mean / (wsum / n + 1e-8)
        dir_loss = dir_sum / (wsum + 1e-8)
    else:
        mag_loss = mag_mean
        dir_loss = dir_sum
    return np.float32(mag_loss + dir_loss)


def kernel(output, target, mask):
    res = _run(np.asarray(output), np.asarray(target), np.asarray(mask))
    return _combine(res)


_TLSIM_NS = None


def timeline_estimate_ns():
    global _TLSIM_NS
    if _TLSIM_NS is None:
        from concourse.timeline_sim import TimelineSim
        _TLSIM_NS = TimelineSim(_build(), trace=False).simulate()
    return _TLSIM_NS


def kernel_timed(output, target, mask):
    res = _run(np.asarray(output), np.asarray(target), np.asarray(mask))
    return _combine(res), timeline_estimate_ns()
